# revision 22
# baseline (speedup 1.0000x reference)
"""Trainium2 Bass kernel for nn_BGFG_XYZT (3-branch NeRF-style MLP blend).

Strategy
--------
Pure data parallel over 8 NeuronCores (points axis). Per core, points are
processed in blocks of 512 with activations laid out [feature, point]
(features on SBUF partitions) so the small MLP layers chain as
stationary-weight matmuls (weights = lhsT, activations = rhs, N = 512).

- Hidden layers run in bf16 (weights + activations), fp32 PSUM accumulate.
- bg/fg branches are packed block-diagonally into single [<=128, <=128]
  matmuls; the actor branch rides in the second half (cols 512:1024) of the
  same fp32 PSUM pair.
- The input transpose ([point, feat] -> [feat, point]) runs on the PE via
  identity matmuls over stacked [128, 99] tiles (xyz|xyzt|cam|views).
- keep_mask is folded into sigma *before* softplus via an accumulated
  mask matmul adding (mask-1)*1e9 to the sigma row (softplus(-1e9) == 0).
- The final sigma-weighted color blend is done with two small fp32
  (float32r) matmuls (F1 replicates sigmas/computes sigma_tot, F2 reduces
  sigma_b*color_b over branches), one sigmoid, one softplus, one
  reciprocal_approx and two elementwise multiplies.
- Output [4, 512] is PE-transposed back to [point, 4] and DMA'd out.

kernel(**inputs) takes the FULL fp32 inputs, shards N=1048576 over 8 cores
(131072 points each), runs the same program SPMD, and concatenates.
"""

import numpy as np
import ml_dtypes

import concourse.bass as bass
import concourse.bacc as bacc
import concourse.mybir as mybir
from concourse.tile import TileContext
from concourse.bass_utils import run_bass_kernel_spmd

DT = mybir.dt
AF = mybir.ActivationFunctionType
BF = ml_dtypes.bfloat16

N_TOTAL = 1_048_576
N_CORES = 8
NC_PTS = N_TOTAL // N_CORES          # 131072 points per core
BLK = 512                            # points per block (one fp32 PSUM bank)
NBLK = NC_PTS // BLK                 # 256 blocks
GROUP = 16                           # blocks per mask-DMA group
BIGB = float(np.float32(BF(1e9)))    # mask bias constant, exactly representable


def _bf(x):
    return np.asarray(x).astype(BF)


def _pack_weights(inp):
    """Host-side packing of all weight/constant DRAM tensors."""
    f32 = np.float32
    g = lambda k: np.asarray(inp[k], f32)

    w = {}
    # s0: K=32 each. bg rows 0:32 -> cols 0:64 ; fg rows 32:64 -> cols 64:128
    W = np.zeros((64, 128), f32)
    W[0:32, 0:64] = g('bg_s0'); W[32:64, 64:128] = g('fg_s0')
    w['W_s0'] = _bf(W)
    W = np.zeros((32, 128), f32)
    W[:, 0:64] = g('actor_s0')
    w['W_s0ac'] = _bf(W)                                   # @ parts 64:96

    W = np.zeros((128, 128), f32)
    W[0:64, 0:64] = g('bg_s1'); W[64:128, 64:128] = g('fg_s1')
    w['W_s1'] = _bf(W)
    W = np.zeros((64, 128), f32)
    W[:, 0:64] = g('actor_s1')
    w['W_s1ac'] = _bf(W)

    # s2 reordered cols: [geo(15), sigma] (junk col dropped) -> M=16 per branch
    perm = list(range(2, 17)) + [0]
    W = np.zeros((128, 32), f32)
    W[0:64, 0:16] = g('bg_s2')[:, perm]
    W[64:128, 16:32] = g('fg_s2')[:, perm]
    w['W_s2'] = _bf(W)
    w['W_s2ac'] = _bf(g('actor_s2')[:, perm])              # [64, 16]

    # sigma mask bias: maskT rows: 0=bg 1=fg 2=ac 3=ones
    W = np.zeros((4, 32), f32)
    W[0, 15] = BIGB; W[3, 15] = -BIGB                      # bg sigma @ psE row 15
    W[1, 31] = BIGB; W[3, 31] = -BIGB                      # fg sigma @ psE row 31
    w['W_sm2'] = _bf(W)
    W = np.zeros((4, 16), f32)
    W[2, 15] = BIGB; W[3, 15] = -BIGB                      # ac sigma @ psF row 15
    w['W_smac'] = _bf(W)

    # c0. gT rows: 0:15 bg geo, 15 sbg, 16:31 fg geo, 31 sfg,
    #              32:47 ac geo, 47 sac
    W = np.zeros((48, 128), f32)
    W[0:15, 0:64] = g('bg_c0')[3:18]                       # bg geo part
    W[16:31, 64:128] = g('fg_c0')
    w['W_c0'] = _bf(W)
    W = np.zeros((3, 128), f32)
    W[:, 0:64] = g('bg_c0')[0:3]
    w['W_c0v'] = _bf(W)                                    # @ parts 96:99
    W = np.zeros((15, 128), f32)
    W[:, 0:64] = g('actor_c0')
    w['W_c0ac'] = _bf(W)                                   # @ parts 32:47

    for li in ('1', '2'):
        W = np.zeros((128, 128), f32)
        W[0:64, 0:64] = g('bg_c' + li); W[64:128, 64:128] = g('fg_c' + li)
        w['W_c' + li] = _bf(W)
        Wa = np.zeros((64, 128), f32)
        Wa[:, 0:64] = g('actor_c' + li)
        w['W_c' + li + 'ac'] = _bf(Wa)

    W = np.zeros((128, 32), f32)                           # zero cols 6:32 defined
    W[0:64, 0:3] = g('bg_c3'); W[64:128, 3:6] = g('fg_c3')
    w['W_c3'] = _bf(W)
    W = np.zeros((64, 9), f32)
    W[:, 0:3] = g('actor_c3')
    w['W_c3ac'] = _bf(W)                                   # psC3 rows 32:41
    # c3 "ones" rider: accumulates into psC3[32:41] -> rows 35:41 = +1e9
    # (tanh(0.5x) -> 1.0); rows 32:35 += 0. Kept to out-range [32:41] so it
    # can never clobber rows whose has_written bits were cleared by a later
    # start=True matmul.
    W = np.zeros((4, 9), f32)
    W[3, 3:9] = BIGB
    w['W_c3on'] = _bf(W)

    # F1: rhs = sig [48, 512] = softplus(gT[0:48]); sigma rows 15/31/47
    # out psR rows: 0:3 sbg, 3:6 sfg, 32:35 sac (aligned w/ Tf),
    #     35:39 stot (+1e-9 via F1b), 39 = 1.0 (F1b), 40 = stot (no eps)
    W = np.zeros((48, 41), f32)
    for c in range(3):
        W[15, 0 + c] = 1.0; W[31, 3 + c] = 1.0; W[47, 32 + c] = 1.0
    for c in list(range(35, 39)) + [40]:
        W[15, c] = 1.0; W[31, c] = 1.0; W[47, c] = 1.0
    w['W_f1'] = _bf(W)
    # F1b rider (rhs = maskT, ones row 33) into psR[32:41]:
    # +1e-9 to stot rows 35:39, +1.0 to row 39, +0 to row 40
    W = np.zeros((4, 41), f32)
    W[3, 35:39] = 1e-9
    W[3, 39] = 1.0
    w['W_f1b'] = _bf(W)

    # F2: rhs = nmt = Tf * psR [41, 512] where Tf = tanh(0.5*psC3):
    # rows 0:6 bg/fg sm_b*t_b (r g b), 32:35 ac, 35:39 stot+1e-9,
    # 39 = 1.0, 40 = stot (no eps).
    # sigmoid(x) = 0.5 + 0.5*tanh(x/2), so numerator_c
    #   = sum_b sm_b*(0.5+0.5 t_bc) = 0.5*stot_noeps + 0.5*sum_b sm_b t_bc
    # out psU rows: 0:3 = color numerators, 3 = stot+1e-9,
    #     32:35 = stot+1e-9 (recip input, 32-aligned), 35 = 1.0 (recip -> 1.0)
    W = np.zeros((41, 36), f32)
    for c in range(3):
        W[c, c] = 0.5; W[3 + c, c] = 0.5; W[32 + c, c] = 0.5
        W[40, c] = 0.5
    W[35, 3] = 1.0
    W[36, 32] = 1.0; W[37, 33] = 1.0; W[38, 34] = 1.0
    W[39, 35] = 1.0
    w['W_f2'] = _bf(W)

    w['id128'] = _bf(np.eye(128, dtype=f32))
    w['id4'] = np.eye(4, dtype=f32)
    return w


# (name, shape, dtype, base_partition_of_data)
_WSPECS = [
    ('W_s0',   (64, 128),  DT.bfloat16, 0),
    ('W_s0ac', (32, 128),  DT.bfloat16, 64),
    ('W_s1',   (128, 128), DT.bfloat16, 0),
    ('W_s1ac', (64, 128),  DT.bfloat16, 0),
    ('W_s2',   (128, 32),  DT.bfloat16, 0),
    ('W_s2ac', (64, 16),   DT.bfloat16, 0),
    ('W_sm2',  (4, 32),    DT.bfloat16, 0),
    ('W_smac', (4, 16),    DT.bfloat16, 0),
    ('W_c0',   (48, 128),  DT.bfloat16, 0),
    ('W_c0v',  (3, 128),   DT.bfloat16, 96),
    ('W_c0ac', (15, 128),  DT.bfloat16, 32),
    ('W_c1',   (128, 128), DT.bfloat16, 0),
    ('W_c1ac', (64, 128),  DT.bfloat16, 0),
    ('W_c2',   (128, 128), DT.bfloat16, 0),
    ('W_c2ac', (64, 128),  DT.bfloat16, 0),
    ('W_c3',   (128, 32),  DT.bfloat16, 0),
    ('W_c3ac', (64, 9),    DT.bfloat16, 0),
    ('W_c3on', (4, 9),     DT.bfloat16, 0),
    ('W_f1',   (48, 41),   DT.bfloat16, 0),
    ('W_f1b',  (4, 41),    DT.bfloat16, 0),
    ('W_f2',   (41, 36),   DT.bfloat16, 0),
    ('id128',  (128, 128), DT.bfloat16, 0),
    ('id4',    (4, 4),     DT.float32, 0),
]


def build_program(nblk=NBLK, group=GROUP):
    npts = nblk * BLK
    nc = bacc.Bacc()

    xs_d = nc.declare_dram_parameter('xs', [npts, 99], DT.bfloat16, isOutput=False)
    mk_d = nc.declare_dram_parameter('maskp', [4, npts], DT.bfloat16, isOutput=False)
    wd = {}
    for name, shape, dt, _base in _WSPECS:
        wd[name] = nc.declare_dram_parameter(name, list(shape), dt, isOutput=False)
    out_d = nc.declare_dram_parameter('out', [npts, 4], DT.float32, isOutput=True)

    f32r = DT.float32r

    with TileContext(nc) as tc:
        from contextlib import ExitStack
        with ExitStack() as ctx:
            consts = ctx.enter_context(tc.tile_pool(name='consts', bufs=1))
            inp = ctx.enter_context(tc.tile_pool(name='inp', bufs=3))
            mpool = ctx.enter_context(tc.tile_pool(name='mask', bufs=2))
            act = ctx.enter_context(tc.tile_pool(name='act', bufs=2))
            psb = ctx.enter_context(tc.tile_pool(name='psb', bufs=2, space='PSUM'))
            pss = ctx.enter_context(tc.tile_pool(name='pss', bufs=3, space='PSUM'))

            # --- load weights once ---
            wt = {}
            for name, shape, dt, base in _WSPECS:
                p, f = shape
                t = consts.tile([base + p, f], dt, tag=name)
                nc.sync.dma_start(out=t[base:base + p, :], in_=wd[name][:])
                wt[name] = t

            def emit_block(b, maskT, k):
                mT = maskT[:, k * BLK:(k + 1) * BLK]

                # input DMA: [512, 99] -> [128 parts, 4 subblocks, 99 feats]
                S = inp.tile([128, 4, 99], DT.bfloat16, tag='S')
                nc.sync.dma_start(
                    out=S[:],
                    in_=xs_d[b * BLK:(b + 1) * BLK, :].rearrange(
                        '(p q) f -> p q f', p=128))

                # PE transpose to [feat, point]
                xTps = pss.tile([99, 512], DT.bfloat16, tag='sm')
                for q in range(4):
                    nc.tensor.transpose(
                        xTps[:, q * 128:(q + 1) * 128], S[:, q, :],
                        wt['id128'][:])
                xT = act.tile([99, 512], DT.bfloat16, tag='xT')
                nc.vector.tensor_copy(xT[:], xTps[:])

                # s0
                psH0 = psb.tile([128, 1024], DT.float32, tag='big')
                nc.tensor.matmul(psH0[:, 0:512], wt['W_s0'][:], xT[0:64, :])
                nc.tensor.matmul(psH0[:, 512:1024], wt['W_s0ac'][64:96, :],
                                 xT[64:96, :])
                h0 = act.tile([128, 1024], DT.bfloat16, tag='h0')
                nc.scalar.activation(h0[:], psH0[:], AF.Relu)

                # s1
                psH1 = psb.tile([128, 1024], DT.float32, tag='big')
                nc.tensor.matmul(psH1[:, 0:512], wt['W_s1'][:], h0[:, 0:512])
                nc.tensor.matmul(psH1[:, 512:1024], wt['W_s1ac'][0:64, :],
                                 h0[0:64, 512:1024])
                h1 = act.tile([128, 1024], DT.bfloat16, tag='h1')
                nc.vector.tensor_scalar_max(h1[:], psH1[:], 0.0)

                # s2 (+ sigma mask bias) -- separate banks for bgfg / ac
                psE = pss.tile([32, 512], DT.float32, tag='sm')
                nc.tensor.matmul(psE[:], wt['W_s2'][:], h1[:, 0:512],
                                 start=True, stop=False)
                nc.tensor.matmul(psE[:], wt['W_sm2'][0:4, :], mT,
                                 start=False, stop=True)
                psF = pss.tile([16, 512], DT.float32, tag='sm')
                nc.tensor.matmul(psF[:], wt['W_s2ac'][0:64, :],
                                 h1[0:64, 512:1024], start=True, stop=False)
                nc.tensor.matmul(psF[:], wt['W_smac'][0:4, :], mT,
                                 start=False, stop=True)

                # gT assembly: geo+sigma rows (bf16); views stay in xT
                gT = act.tile([48, 512], DT.bfloat16, tag='gT')
                nc.vector.tensor_copy(gT[0:32, :], psE[:])
                nc.scalar.copy(gT[32:48, :], psF[:])

                # c0 (bg views part rides as an accumulate from xT[96:99])
                psC0 = psb.tile([128, 1024], DT.float32, tag='big')
                nc.tensor.matmul(psC0[:, 0:512], wt['W_c0'][:], gT[0:48, :],
                                 start=True, stop=False)
                nc.tensor.matmul(psC0[:, 0:512], wt['W_c0v'][96:99, :],
                                 xT[96:99, :], start=False, stop=True,
                                 tile_position=(96, 0))
                nc.tensor.matmul(psC0[:, 512:1024], wt['W_c0ac'][32:47, :],
                                 gT[32:47, :])
                c0 = act.tile([128, 1024], DT.bfloat16, tag='c0')
                nc.scalar.activation(c0[:], psC0[:], AF.Relu)

                # c1
                psC1 = psb.tile([128, 1024], DT.float32, tag='big')
                nc.tensor.matmul(psC1[:, 0:512], wt['W_c1'][:], c0[:, 0:512])
                nc.tensor.matmul(psC1[:, 512:1024], wt['W_c1ac'][0:64, :],
                                 c0[0:64, 512:1024])
                c1 = act.tile([128, 1024], DT.bfloat16, tag='c1')
                nc.vector.tensor_scalar_max(c1[:], psC1[:], 0.0)

                # c2
                psC2 = psb.tile([128, 1024], DT.float32, tag='big')
                nc.tensor.matmul(psC2[:, 0:512], wt['W_c2'][:], c1[:, 0:512])
                nc.tensor.matmul(psC2[:, 512:1024], wt['W_c2ac'][0:64, :],
                                 c1[0:64, 512:1024])
                c2 = act.tile([128, 1024], DT.bfloat16, tag='c2')
                nc.scalar.activation(c2[:], psC2[:], AF.Relu)

                # c3 -> one bank: rows 0:32 bgfg (cols 6:32 zero), 32:35 ac,
                # then the ones-rider sets rows 35:41 = 1e9 (tanh -> 1.0)
                psC3 = pss.tile([41, 512], DT.float32, tag='sm')
                nc.tensor.matmul(psC3[0:32, :], wt['W_c3'][:], c2[:, 0:512])
                nc.tensor.matmul(psC3[32:41, :], wt['W_c3ac'][0:64, :],
                                 c2[0:64, 512:1024], start=True, stop=False)
                nc.tensor.matmul(psC3[32:41, :], wt['W_c3on'][0:4, :], mT,
                                 start=False, stop=True)

                # softplus(x) = ln(1 + exp(x)); tanh gives sigmoid via F2.
                # All ACT funcs (exp, ln, tanh, relu, copy) live in ONE
                # table set -> no per-block table reloads.
                ex = act.tile([48, 512], DT.bfloat16, tag='ex')
                nc.scalar.activation(ex[:], gT[0:48, :], AF.Exp)
                sig = act.tile([48, 512], DT.bfloat16, tag='sig')
                nc.scalar.activation(sig[:], ex[:], AF.Ln, bias=1.0)
                Tf = act.tile([41, 512], DT.bfloat16, tag='Tf')
                nc.scalar.activation(Tf[:], psC3[:], AF.Tanh, scale=0.5)

                # F1: replicate sigmas + sigma_tot; F1b adds 1e-9 and the
                # 1.0 passthrough row 39
                psR = pss.tile([41, 512], DT.float32, tag='sm')
                nc.tensor.matmul(psR[0:41, :], wt['W_f1'][:], sig[:],
                                 start=True, stop=False)
                nc.tensor.matmul(psR[0:41, :], wt['W_f1b'][0:4, :], mT,
                                 start=False, stop=True)
                nmt = act.tile([41, 512], DT.bfloat16, tag='nmt')
                nc.vector.tensor_mul(nmt[:], Tf[:], psR[:])

                # F2: reduce over branches
                psU = pss.tile([36, 512], DT.float32, tag='sm')
                nc.tensor.matmul(psU[:], wt['W_f2'][:], nmt[:])

                lg = act.tile([4, 512], DT.float32, tag='lg')
                nc.scalar.activation(lg[:], psU[32:36, :], AF.Ln)
                rT = act.tile([4, 512], DT.float32, tag='rT')
                nc.scalar.activation(rT[:], lg[:], AF.Exp, scale=-1.0)
                oT = act.tile([4, 512], DT.float32, tag='oT')
                nc.vector.tensor_mul(oT[:], psU[0:4, :], rT[:])

                # transpose back to [point, 4] and store
                psV = pss.tile([128, 16], DT.float32, tag='sm')
                for j in range(4):
                    nc.tensor.transpose(psV[:, j * 4:(j + 1) * 4],
                                        oT[:, j * 128:(j + 1) * 128],
                                        wt['id4'][:])
                oS = act.tile([128, 16], DT.float32, tag='oS')
                nc.scalar.copy(oS[:], psV[:])
                nc.sync.dma_start(
                    out=out_d[b * BLK:(b + 1) * BLK, :].rearrange(
                        '(p q) c -> p q c', p=128),
                    in_=oS[:].rearrange('p (q c) -> p q c', q=4))

            for g in range(nblk // group):
                maskT = mpool.tile([4, group * BLK], DT.bfloat16, tag='maskT')
                c0_ = g * group * BLK
                c1_ = (g + 1) * group * BLK
                nc.sync.dma_start(out=maskT[:], in_=mk_d[:, c0_:c1_])
                for k in range(group):
                    emit_block(g * group + k, maskT, k)

    nc.finalize()
    return nc


def _prep_core_inputs(inputs, lo, hi, wpacked):
    """Host-side prep of one core's input map."""
    f32 = np.float32
    xs = np.concatenate([
        np.asarray(inputs['embedded_xyz'][lo:hi], f32),
        np.asarray(inputs['embedded_xyzt'][lo:hi], f32),
        np.asarray(inputs['embedded_xyzt_cam'][lo:hi], f32),
        np.asarray(inputs['input_views'][lo:hi], f32),
    ], axis=1)                                    # [npts, 99]
    npts = hi - lo
    # mask rows (bg, fg, ac, ones), block-permuted to match the transpose
    # layout: col (b*512 + q*128 + p) <- point (b*512 + p*4 + q)
    mk = np.empty((4, npts), f32)
    for r, key in enumerate(('keep_mask_bg', 'keep_mask_fg', 'keep_mask_actor')):
        m = np.asarray(inputs[key][lo:hi], f32).reshape(-1, 128, 4)
        mk[r] = m.transpose(0, 2, 1).reshape(-1)
    mk[3] = 1.0
    im = {'xs': _bf(xs), 'maskp': _bf(mk)}
    im.update(wpacked)
    return im


_CACHED_NC = None


def kernel(**inputs):
    global _CACHED_NC
    wpacked = _pack_weights(inputs)
    if _CACHED_NC is None:
        _CACHED_NC = build_program()
    nc = _CACHED_NC

    in_maps = []
    for c in range(N_CORES):
        in_maps.append(_prep_core_inputs(inputs, c * NC_PTS, (c + 1) * NC_PTS,
                                         wpacked))
    res = run_bass_kernel_spmd(nc, in_maps, list(range(N_CORES))).results
    out = np.concatenate([np.asarray(res[c]['out']) for c in range(N_CORES)],
                         axis=0)
    return out.astype(np.float32)
